# revision 28
# baseline (speedup 1.0000x reference)
"""Trainium2 Bass kernel for nn_GAT_sin_35399120453979.

Two GAT layers over a fixed sparse graph (4096 nodes, ~2% density, self
loops), batch 8.  Data-parallel over batch: one batch element per
NeuronCore, graph replicated (shipped as bf16 — 0/1 values are exact).

Per core, per layer (flash-attention style streaming softmax):
  h      = x @ W.T                       (fp32 PE matmuls)
  hi/lo  = fp16 split of h               (products are exact in fp32 PSUM)
  s      = h @ h.T   via hi*hi + hi*lo + lo*hi   (layer 2: K-stacked 2-pass)
  t      = s * g                         (DVE tensor_mul, PSUM -> SBUF)
  -m     = reduce_max(t, negate=True), merged into a running negated max
  p      = exp(t - m)                    (ACT, fused row-sum accum -> Z)
           masked entries have t=0 and the final m >= |h_i|^2 >> 103, so
           exp(0 - m) underflows to exactly 0.0: no re-mask needed.  Chunks
           processed before the row max is seen are zeroed by the standard
           flash rescale o *= exp(m_old - m_new).
  o      = sum_j p_ij h_j  via PE transposes of p + PV matmuls (fp16,
           packed [h_hi | h_lo] rhs, halves summed at the end)
  out    = o / Z + b       (+ ReLU after layer 1)

Two row blocks are emitted interleaved (generator-based) to hide the
per-chunk dependency-chain latency; g-tile DMAs alternate between the SP
and GPSIMD DMA paths.
"""

import sys

sys.path.insert(0, "/opt/trn_rl_repo")

import numpy as np
import ml_dtypes

B, N, DIN, DH, DO = 8, 4096, 64, 128, 64
CH = 1024                    # softmax chunk columns
GROUP = 2                    # row blocks interleaved in flight
NB = N // 128                # node blocks
NCH = N // CH                # chunks per block row
N_CORES = 8

_CACHE: dict = {}


def _build_program(reps=1):
    import concourse.bacc as bacc
    import concourse.mybir as mybir
    import concourse.tile as tile
    from concourse.masks import make_identity

    dt = mybir.dt
    f32, f16, bf16 = dt.float32, dt.float16, dt.bfloat16
    AF = mybir.ActivationFunctionType
    OP = mybir.AluOpType

    nc = bacc.Bacc("TRN2", target_bir_lowering=False, debug=False,
                   num_devices=N_CORES)

    x_d = nc.dram_tensor("x", [N, DIN], f32, kind="ExternalInput").ap()
    g_d = nc.dram_tensor("g", [N, N], bf16, kind="ExternalInput").ap()
    w1t_d = nc.dram_tensor("w1t", [DIN, DH], f32, kind="ExternalInput").ap()
    w2t_d = nc.dram_tensor("w2t", [DH, DO], f32, kind="ExternalInput").ap()
    b1_d = nc.dram_tensor("b1row", [1, DH], f32, kind="ExternalInput").ap()
    b2_d = nc.dram_tensor("b2row", [1, DO], f32, kind="ExternalInput").ap()
    out_d = nc.dram_tensor("out", [N, DO], f32, kind="ExternalOutput").ap()

    with tile.TileContext(nc) as tc:
        rep_ctx = tc.For_i(0, reps, 1) if reps > 1 else None
        if rep_ctx is not None:
            rep_ctx.__enter__()
        with (
            tc.tile_pool(name="const", bufs=1) as const,
            tc.tile_pool(name="strip", bufs=1) as strip,
            tc.tile_pool(name="gp", bufs=6) as gp,
            tc.tile_pool(name="tp", bufs=9) as tp,
            tc.tile_pool(name="ppool", bufs=3) as ppool,
            tc.tile_pool(name="ptp", bufs=3) as ptp,
            tc.tile_pool(name="stats", bufs=48) as stats,
            tc.tile_pool(name="outp", bufs=4) as outp,
            tc.tile_pool(name="sps", bufs=2, space="PSUM") as sps,
            tc.tile_pool(name="tps", bufs=2, space="PSUM") as tps,
            tc.tile_pool(name="ops", bufs=2, space="PSUM") as ops,
        ):
            id32 = const.tile([128, 128], f32, tag="id32")
            make_identity(nc, id32[:, :])
            id16 = const.tile([128, 128], f16, tag="id16")
            make_identity(nc, id16[:, :])
            ones = const.tile([1, 128], f32, tag="ones")
            nc.gpsimd.memset(ones[:, :], 1.0)

            w1t = const.tile([DIN, DH], f32, tag="w1t")
            nc.sync.dma_start(w1t[:, :], w1t_d[:, :])
            w2t = const.tile([DH, DO], f32, tag="w2t")
            nc.sync.dma_start(w2t[:, :], w2t_d[:, :])
            b1r = const.tile([1, DH], f32, tag="b1r")
            nc.sync.dma_start(b1r[:, :], b1_d[:, :])
            b2r = const.tile([1, DO], f32, tag="b2r")
            nc.sync.dma_start(b2r[:, :], b2_d[:, :])

            # broadcast biases across partitions via ones-matmul
            b1b = const.tile([128, DH], f32, tag="b1b")
            b2b = const.tile([128, DO], f32, tag="b2b")
            ps = sps.tile([128, CH], f32, tag="s")
            nc.tensor.matmul(ps[:, :DH], ones[:1, :], b1r[:1, :], start=True, stop=True)
            nc.scalar.activation(b1b[:, :], ps[:, :DH], AF.Copy)
            ps = sps.tile([128, CH], f32, tag="s")
            nc.tensor.matmul(ps[:, :DO], ones[:1, :], b2r[:1, :], start=True, stop=True)
            nc.scalar.activation(b2b[:, :], ps[:, :DO], AF.Copy)

            # ---- phase A1: x, xT, h1 in both layouts (fp16 hi/lo) ----
            xn = strip.tile([128, NB * DIN], f32, tag="xn")       # x tiles, node-major
            for i in range(NB):
                nc.sync.dma_start(xn[:, i * DIN:(i + 1) * DIN],
                                  x_d[i * 128:(i + 1) * 128, :])
            xT = strip.tile([DIN, N], f32, tag="xT")
            for i in range(NB):
                pst = tps.tile([128, CH // 2], f32, tag="t")
                nc.tensor.transpose(pst[:DIN, :128], xn[:, i * DIN:(i + 1) * DIN], id32[:, :])
                nc.scalar.activation(xT[:, i * 128:(i + 1) * 128], pst[:DIN, :128], AF.Copy)

            h1T_hi = strip.tile([DH, N], f16, tag="h1T_hi")
            h1T_lo = strip.tile([DH, N], f16, tag="h1T_lo")
            for c in range(N // 512):
                ps = sps.tile([128, CH], f32, tag="s")
                nc.tensor.matmul(ps[:, :512], w1t[:, :], xT[:, c * 512:(c + 1) * 512],
                                 start=True, stop=True)
                nc.scalar.activation(h1T_hi[:, c * 512:(c + 1) * 512], ps[:, :512], AF.Copy)
                nc.vector.tensor_sub(h1T_lo[:, c * 512:(c + 1) * 512], ps[:, :512],
                                     h1T_hi[:, c * 512:(c + 1) * 512])

            # natural-layout h1, packed [hi_j (DH) | lo_j (DH)] per node block j
            h1n_packed = strip.tile([128, NB * 2 * DH], f16, tag="h1n_packed")
            for j in range(NB):
                ps = sps.tile([128, CH], f32, tag="s")
                nc.tensor.matmul(ps[:, :DH], xT[:, j * 128:(j + 1) * 128], w1t[:, :],
                                 start=True, stop=True)
                nc.scalar.activation(h1n_packed[:, j * 2 * DH:j * 2 * DH + DH],
                                     ps[:, :DH], AF.Copy)
                nc.vector.tensor_sub(h1n_packed[:, j * 2 * DH + DH:(j + 1) * 2 * DH],
                                     ps[:, :DH],
                                     h1n_packed[:, j * 2 * DH:j * 2 * DH + DH])

            x2T = strip.tile([128, N], f32, tag="x2T")

            def attention_block_gen(i, qk_emit, hn, hw, dh, bbt, relu, out_sink):
                """One 128-row block of masked attention + aggregation.

                Generator: yields after each chunk so two blocks can be
                emitted interleaved (hides per-chunk dependency latency).
                """
                o_ps = ops.tile([128, 2 * dh], f32, tag="o")
                nmall = stats.tile([128, NCH], f32, tag="nm4")
                zall = stats.tile([128, NCH], f32, tag="z4")
                t_tiles = []
                for c in range(NCH):
                    g_t = gp.tile([128, CH], bf16, tag="g")
                    dma_eng = nc.sync if (i + c) % 2 == 0 else nc.gpsimd
                    dma_eng.dma_start(g_t[:, :],
                                      g_d[i * 128:(i + 1) * 128, c * CH:(c + 1) * CH])
                    s_ps = sps.tile([128, CH], f32, tag="s")
                    qk_emit(i, c, s_ps)
                    yield
                    t_t = tp.tile([128, CH], f32, tag="t")
                    t_tiles.append(t_t)
                    nc.vector.tensor_mul(t_t[:, :], s_ps[:, :], g_t[:, :])
                    nc.vector.reduce_max(nmall[:, c:c + 1], t_t[:, :],
                                         axis=mybir.AxisListType.X, negate=True)
                    yield
                # strip max (negated): min over the per-chunk negated maxes
                negm = stats.tile([128, 1], f32, tag="st")
                nc.vector.tensor_reduce(negm[:, 0:1], nmall[:, :],
                                        axis=mybir.AxisListType.X, op=OP.min)
                # phase 2: exp + transpose + PV per chunk, no rescales needed
                for c in range(NCH):
                    p_t = ppool.tile([128, CH], f16, tag="p")
                    nc.scalar.activation(p_t[:, :], t_tiles[c][:, :], AF.Exp,
                                         bias=negm[:, 0:1],
                                         accum_out=zall[:, c:c + 1])
                    pT_ps = tps.tile([128, CH], f16, tag="t")
                    for k in range(CH // 128):
                        nc.tensor.transpose(pT_ps[:, k * 128:(k + 1) * 128],
                                            p_t[:, k * 128:(k + 1) * 128], id16[:, :])
                    pT_t = ptp.tile([128, CH], f16, tag="pT")
                    nc.scalar.activation(pT_t[:, :], pT_ps[:, :], AF.Copy)
                    for k in range(CH // 128):
                        j = c * (CH // 128) + k
                        first = (c == 0 and k == 0)
                        last = (c == NCH - 1 and k == CH // 128 - 1)
                        nc.tensor.matmul(o_ps[:, :], pT_t[:, k * 128:(k + 1) * 128],
                                         hn[:, j * hw:j * hw + 2 * dh],
                                         start=first, stop=last,
                                         skip_group_check=True)
                    yield
                z_sum = stats.tile([128, 1], f32, tag="st")
                nc.vector.tensor_reduce(z_sum[:, 0:1], zall[:, :],
                                        axis=mybir.AxisListType.X, op=OP.add)
                recip = stats.tile([128, 1], f32, tag="st")
                nc.vector.reciprocal(recip[:, 0:1], z_sum[:, 0:1])
                o_sb = outp.tile([128, 2 * dh], f32, tag="osb")
                nc.scalar.activation(o_sb[:, :], o_ps[:, :], AF.Copy)
                o_sum = outp.tile([128, dh], f32, tag="osum")
                nc.vector.tensor_add(o_sum[:, :], o_sb[:, 0:dh], o_sb[:, dh:2 * dh])
                o_t = outp.tile([128, dh], f32, tag="ot")
                nc.vector.scalar_tensor_tensor(o_t[:, :], o_sum[:, :], recip[:, 0:1],
                                               bbt[:, :dh], op0=OP.mult, op1=OP.add)
                if relu:
                    o_r = outp.tile([128, dh], f32, tag="ot")
                    nc.vector.tensor_scalar_max(o_r[:, :], o_t[:, :], 0.0)
                    o_t = o_r
                out_sink(i, o_t)

            # ---- layer 1 ----
            def qk1(i, c, s_ps):
                wi_hi = h1T_hi[:, i * 128:(i + 1) * 128]
                wi_lo = h1T_lo[:, i * 128:(i + 1) * 128]
                for w, rhs_strip, first, last in ((wi_hi, h1T_hi, True, False),
                                                  (wi_hi, h1T_lo, False, False),
                                                  (wi_lo, h1T_hi, False, True)):
                    for sc in range(CH // 512):
                        col = c * CH + sc * 512
                        nc.tensor.matmul(s_ps[:, sc * 512:(sc + 1) * 512], w,
                                         rhs_strip[:, col:col + 512],
                                         start=first, stop=last)

            def sink1(i, o_t):
                pst = tps.tile([128, CH // 2], f32, tag="t")
                nc.tensor.transpose(pst[:, 0:128], o_t[:, :], id32[:, :])
                nc.scalar.activation(x2T[:, i * 128:(i + 1) * 128], pst[:, 0:128], AF.Copy)

            def drive_blocks(mk_gen, group):
                idx = 0
                while idx < NB:
                    n = min(group, NB - idx)
                    gens = [mk_gen(idx + k) for k in range(n)]
                    done = [False] * n
                    while not all(done):
                        for gi in range(n):
                            if not done[gi]:
                                done[gi] = next(gens[gi], "end") == "end"
                    idx += n

            drive_blocks(lambda i: attention_block_gen(
                i, qk1, h1n_packed, 2 * DH, DH, b1b, True, sink1), GROUP)


            # ---- phase A2: h2 in both layouts, K-stacked QK operands ----
            hi2T = strip.tile([DO, N], f16, tag="hi2T")
            lo2T = strip.tile([DO, N], f16, tag="lo2T")
            for c in range(N // 512):
                ps = sps.tile([128, CH], f32, tag="s")
                nc.tensor.matmul(ps[:DO, :512], w2t[:, :], x2T[:, c * 512:(c + 1) * 512],
                                 start=True, stop=True)
                nc.scalar.activation(hi2T[:, c * 512:(c + 1) * 512], ps[:DO, :512], AF.Copy)
                nc.vector.tensor_sub(lo2T[:, c * 512:(c + 1) * 512], ps[:DO, :512],
                                     hi2T[:, c * 512:(c + 1) * 512])

            st2 = strip.tile([128, N], f16, tag="st2")    # [hi ; lo]
            hh2 = strip.tile([128, N], f16, tag="hh2")    # [hi ; hi]
            h02 = strip.tile([128, N], f16, tag="h02")    # [hi ; 0]
            ll2 = strip.tile([128, N], f16, tag="ll2")    # [lo ; lo]
            nc.scalar.activation(st2[0:DO, :], hi2T[:, :], AF.Copy)
            nc.vector.tensor_copy(st2[DO:128, :], lo2T[:, :])
            nc.scalar.activation(hh2[0:DO, :], hi2T[:, :], AF.Copy)
            nc.vector.tensor_copy(hh2[DO:128, :], hi2T[:, :])
            nc.scalar.activation(h02[0:DO, :], hi2T[:, :], AF.Copy)
            nc.gpsimd.memset(h02[DO:128, :], 0.0)
            nc.scalar.activation(ll2[0:DO, :], lo2T[:, :], AF.Copy)
            nc.vector.tensor_copy(ll2[DO:128, :], lo2T[:, :])

            h2n = strip.tile([128, NB * 2 * DO], f16, tag="h2n")  # [hi_j | lo_j]
            for j in range(NB):
                ps = sps.tile([128, CH], f32, tag="s")
                nc.tensor.matmul(ps[:, :DO], x2T[:, j * 128:(j + 1) * 128], w2t[:, :],
                                 start=True, stop=True)
                nc.scalar.activation(h2n[:, j * 2 * DO:j * 2 * DO + DO], ps[:, :DO], AF.Copy)
                nc.vector.tensor_sub(h2n[:, j * 2 * DO + DO:(j + 1) * 2 * DO], ps[:, :DO],
                                     h2n[:, j * 2 * DO:j * 2 * DO + DO])

            # ---- layer 2 ----
            def qk2(i, c, s_ps):
                w_st = st2[:, i * 128:(i + 1) * 128]
                w_h0 = h02[:, i * 128:(i + 1) * 128]
                for w, rhs_strip, first, last in ((w_st, hh2, True, False),
                                                  (w_h0, ll2, False, True)):
                    for sc in range(CH // 512):
                        col = c * CH + sc * 512
                        nc.tensor.matmul(s_ps[:, sc * 512:(sc + 1) * 512], w,
                                         rhs_strip[:, col:col + 512],
                                         start=first, stop=last)

            def sink2(i, o_t):
                nc.sync.dma_start(out_d[i * 128:(i + 1) * 128, :], o_t[:, :])

            drive_blocks(lambda i: attention_block_gen(
                i, qk2, h2n, 2 * DO, DO, b2b, False, sink2), GROUP)


        if rep_ctx is not None:
            rep_ctx.__exit__(None, None, None)
    nc.compile()
    return nc


def _get_nc(reps=1):
    key = f"nc{reps}"
    if key not in _CACHE:
        _CACHE[key] = _build_program(reps)
    return _CACHE[key]


def _make_in_maps(flow_x, graph, W1, b1, W2, b2):
    bf16 = ml_dtypes.bfloat16
    g16 = np.ascontiguousarray(graph.astype(bf16))
    w1t = np.ascontiguousarray(W1.T.astype(np.float32))
    w2t = np.ascontiguousarray(W2.T.astype(np.float32))
    b1r = np.ascontiguousarray(b1.reshape(1, DH).astype(np.float32))
    b2r = np.ascontiguousarray(b2.reshape(1, DO).astype(np.float32))
    in_maps = []
    for c in range(N_CORES):
        in_maps.append({
            "x": np.ascontiguousarray(flow_x[c].astype(np.float32)),
            "g": g16,
            "w1t": w1t,
            "w2t": w2t,
            "b1row": b1r,
            "b2row": b2r,
        })
    return in_maps


def kernel(flow_x, graph, W1, b1, W2, b2):
    flow_x = np.asarray(flow_x)
    graph = np.asarray(graph)
    W1 = np.asarray(W1)
    b1 = np.asarray(b1)
    W2 = np.asarray(W2)
    b2 = np.asarray(b2)

    from concourse import bass_utils

    nc = _get_nc()
    in_maps = _make_in_maps(flow_x, graph, W1, b1, W2, b2)
    res = None
    for attempt in range(3):
        try:
            res = bass_utils.run_bass_kernel_spmd(
                nc, in_maps, core_ids=list(range(N_CORES)))
            break
        except Exception:
            # a previously-crashed kernel can leave the device wedged for one
            # run; retry once or twice before giving up
            if attempt == 2:
                raise
    out = np.stack([res.results[c]["out"] for c in range(N_CORES)], axis=0)
    return out[:, :, None, :].astype(np.float32)
